# revision 11
# baseline (speedup 1.0000x reference)
"""CaptioningRNN forward loss on 8 Trainium2 NeuronCores.

Data-parallel over N: each core handles 16 of the 128 sequences end-to-end
(embed gather, xW precompute, sequential RNN scan, vocab scores + fused
softmax-CE partial sums). Per-(token, vocab-pair-tile) exp-sums and per-token
target scores are DMA'd out; the tiny final log/mask/sum reduction across
vocab tiles and cores happens on the host.

Speed structure:
  - vocab projection runs in fp8e4 DoubleRow (2 contraction rows/cycle);
    W_vocab is converted+interleaved offline, h is converted on-chip.
  - the RNN scan also runs fp8 DoubleRow against an interleaved Wh.
  - score tiles are processed in PAIRS (two PSUM banks) so one exp covers
    1024 columns — ACT op count halves; ACT is the #2 engine.
  - target scores are NOT extracted from the score tiles: a separate tiny
    DoubleRow matmul against host-gathered W_vocab[:, y] columns plus a
    diagonal-extract reduce gives them directly (saves ~84us of DVE).
  - W_vocab tiles stream via a deep pool so DMA hides under the scan;
    score pairs for finished token groups are interleaved into the scan's
    tanh gaps.
  - PSUM pools are era-scoped: xW pool closes after the last xW group, the
    scan pools close before phase 3, so phase 3 gets a 3-deep pair pipeline.

Problem shapes (hardcoded): N=128, T=33 (32 steps), Dfeat=512, W=512,
H=1024, V=16384.
"""
import os
import numpy as np
import ml_dtypes
import concourse.bass as bass
import concourse.tile as tile
from concourse import bacc, mybir
from concourse.bass_utils import run_bass_kernel_spmd
from concourse.masks import make_identity
from contextlib import ExitStack

dt = mybir.dt
AF = mybir.ActivationFunctionType
OP = mybir.AluOpType
E4 = ml_dtypes.float8_e4m3fn

N_CORES = 8
NL = 16          # sequences per core
T_STEPS = 32     # scan steps (T-1)
DF = 512         # feature dim
WD = 512         # word vec dim
H = 1024         # hidden dim
V = 16384        # vocab
NTOK = NL * T_STEPS          # 512 tokens per core (t-major: tok = t*16 + n)
NG = NTOK // 128             # 4 groups of 128 tokens
NJ = V // 512                # 32 vocab column tiles
NJ2 = NJ // 2                # 16 vocab pair tiles (1024 cols each)
KC2 = H // 256               # 4 DoubleRow contraction chunks over H
KC_W = WD // 128             # 4 contraction chunks over W

_nc_cache = {}


def build_program(with_bias=False):
    if with_bias in _nc_cache:
        return _nc_cache[with_bias]
    wv_bufs = int(os.environ.get("KWV_BUFS", "96"))
    nc = bacc.Bacc("TRN2", target_bir_lowering=False, debug=False,
                   num_devices=N_CORES)

    # ---- DRAM parameters (per-core shards / replicated weights) ----
    feat_d = nc.dram_tensor("features", [NL, DF], dt.float32, kind="ExternalInput")
    tok_d = nc.dram_tensor("tok", [128, NG], dt.int32, kind="ExternalInput")
    wembed_d = nc.dram_tensor("W_embed", [V, WD], dt.float32, kind="ExternalInput")
    wproj_d = nc.dram_tensor("W_proj", [DF, H], dt.float32r, kind="ExternalInput")
    wx_d = nc.dram_tensor("Wx", [WD, H], dt.float32r, kind="ExternalInput")
    wh_d = nc.dram_tensor("Wh8", [KC2, 128, 2, H], dt.float8e4, kind="ExternalInput")
    wv_d = nc.dram_tensor("WV8", [KC2, NJ, 128, 2, 512], dt.float8e4,
                          kind="ExternalInput")
    wvy_d = nc.dram_tensor("WVY8", [NG, 128, KC2, 2, 128], dt.float8e4,
                           kind="ExternalInput")
    if with_bias:
        bproj_d = nc.dram_tensor("b_proj", [1, H], dt.float32r, kind="ExternalInput")
        brnn_d = nc.dram_tensor("b_rnn", [1, H], dt.float32r, kind="ExternalInput")
        bvoc_d = nc.dram_tensor("b_vocab", [NJ, 512], dt.float32r,
                                kind="ExternalInput")
    s_out_d = nc.dram_tensor("s_out", [128, NG, NJ2], dt.float32,
                             kind="ExternalOutput")
    t_out_d = nc.dram_tensor("t_out", [128, NG], dt.float32,
                             kind="ExternalOutput")

    with tile.TileContext(nc) as tc, ExitStack() as ctx:
        const = ctx.enter_context(tc.tile_pool(name="const", bufs=1))
        acts = ctx.enter_context(tc.tile_pool(name="acts", bufs=1))
        wts = ctx.enter_context(tc.tile_pool(name="wts", bufs=1))
        scr = ctx.enter_context(tc.tile_pool(name="scr", bufs=2))
        wvp = ctx.enter_context(tc.tile_pool(name="wvp", bufs=wv_bufs))
        stack_scan = ctx.enter_context(ExitStack())
        stack_xw = ctx.enter_context(ExitStack())
        psA = stack_scan.enter_context(tc.tile_pool(name="psA", bufs=2,
                                                    space="PSUM"))
        psB = stack_scan.enter_context(tc.tile_pool(name="psB", bufs=1,
                                                    space="PSUM"))
        psC = stack_xw.enter_context(tc.tile_pool(name="psC", bufs=3,
                                                  space="PSUM"))

        # ---- constants / small inputs (issued before the big streams) ----
        ident128 = const.tile([128, 128], dt.float32)
        make_identity(nc, ident128[:])
        ident16 = const.tile([16, 16], dt.float32)
        make_identity(nc, ident16[:])
        ident128r = const.tile([128, 128], dt.float32r)
        nc.vector.tensor_copy(ident128r[:], ident128[:])
        if with_bias:
            ones16 = const.tile([1, 16], dt.float32r)
            nc.vector.memset(ones16[:].bitcast(dt.float32), 1.0)
            ones128 = const.tile([1, 128], dt.float32r)
            nc.vector.memset(ones128[:].bitcast(dt.float32), 1.0)

        tok_t = const.tile([128, NG], dt.int32)
        nc.sync.dma_start(tok_t[:], tok_d.ap())
        feat_t = const.tile([NL, DF], dt.float32)
        nc.sync.dma_start(feat_t[:], feat_d.ap())
        if with_bias:
            bproj_t = const.tile([1, H], dt.float32r)
            nc.sync.dma_start(bproj_t[:], bproj_d.ap())
            brnn_t = const.tile([1, H], dt.float32r)
            nc.sync.dma_start(brnn_t[:], brnn_d.ap())

        # weight loads (critical before scan / phase1 matmuls): W_proj feeds
        # h0 immediately; Wx feeds xW g0; Wh is needed from scan step 1.
        wp_t = wts.tile([128, KC_W, H], dt.float32r)
        for kc in range(KC_W):
            nc.sync.dma_start(wp_t[:, kc, :], wproj_d.ap()[kc * 128:(kc + 1) * 128, :])
        wx_t = wts.tile([128, KC_W, H], dt.float32r)
        for kc in range(KC_W):
            nc.sync.dma_start(wx_t[:, kc, :], wx_d.ap()[kc * 128:(kc + 1) * 128, :])
        wh_t = wts.tile([128, KC2, 2, H], dt.float8e4)
        for kc2 in range(KC2):
            nc.sync.dma_start(wh_t[:, kc2], wh_d.ap()[kc2])
        wvy_t = wts.tile([128, NG, KC2, 2, 128], dt.float8e4)
        for m in range(NG):
            nc.sync.dma_start(wvy_t[:, m], wvy_d.ap()[m])

        # embed gather for all 4 token groups (runs on its own queues)
        x_all = wts.tile([128, NG, WD], dt.float32)
        for g in range(NG):
            nc.gpsimd.indirect_dma_start(
                out=x_all[:, g, :], out_offset=None,
                in_=wembed_d.ap(),
                in_offset=bass.IndirectOffsetOnAxis(ap=tok_t[:, g:g + 1], axis=0),
            )

        # W_vocab fp8 tile stream: deep pool; j-major, kc2 inner.
        wv_tiles = {}

        def load_wv(j):
            for kc2 in range(KC2):
                t_ = wvp.tile([128, 2, 512], dt.float8e4, tag="wv")
                nc.sync.dma_start(t_[:], wv_d.ap()[kc2, j])
                wv_tiles[(j, kc2)] = t_
            if with_bias:
                b_ = wvp.tile([1, 512], dt.float32r, tag="bv")
                nc.sync.dma_start(b_[:], bvoc_d.ap()[j:j + 1, :])
                wv_tiles[(j, "b")] = b_

        for j in range(NJ):
            load_wv(j)

        # ---- persistent activations ----
        # transposed hidden states, fp8, DoubleRow layout, one tile per token
        # group: hT8m[m][p, kc2, i, c] = h[tok m*128+c, h=(2*kc2+i)*128+p];
        # hT80 holds h0 (16 cols).
        hT8m = [acts.tile([128, KC2, 2, 128], dt.float8e4, tag=f"hT8m{m}",
                          name=f"hT8m{m}") for m in range(NG)]
        hT80 = acts.tile([128, KC2, 2, NL], dt.float8e4)
        xw_all = acts.tile([128, NG, H], dt.float32r)      # x @ Wx (+ b)
        s_cols = acts.tile([128, NG, NJ2], dt.float32)     # exp-sum partials

        def hT_lhs(kc2, c0, w):
            if c0 >= NTOK:
                return hT80[:, kc2, :, :w]
            m, off = c0 // 128, c0 % 128
            return hT8m[m][:, kc2, :, off:off + w]

        # ---- phase 1: h0 = features @ W_proj (+ b_proj) ----
        featT = scr.tile([128, KC_W, 16], dt.float32r, tag="featT")
        ps_t0 = psB.tile([128, KC_W, 16], dt.float32, space="PSUM", tag="tr")
        for kc in range(KC_W):
            nc.tensor.transpose(out=ps_t0[:, kc, :],
                                in_=feat_t[:, kc * 128:(kc + 1) * 128],
                                identity=ident16[:])
        nc.vector.tensor_copy(featT[:], ps_t0[:])
        ps_h = psA.tile([NL, H], dt.float32, space="PSUM", tag="scan")
        for hf in range(2):
            for kc in range(KC_W):
                nc.tensor.matmul(
                    out=ps_h[:, hf * 512:(hf + 1) * 512],
                    lhsT=featT[:, kc, :],
                    rhs=wp_t[:, kc, hf * 512:(hf + 1) * 512],
                    start=(kc == 0), stop=(not with_bias and kc == KC_W - 1))
            if with_bias:
                nc.tensor.matmul(out=ps_h[:, hf * 512:(hf + 1) * 512],
                                 lhsT=ones16[:], rhs=bproj_t[:, hf * 512:(hf + 1) * 512],
                                 start=False, stop=True)
        h_scr0 = scr.tile([NL, H], dt.float32, tag="hscr")
        nc.scalar.copy(h_scr0[:], ps_h[:])

        def emit_h_transpose_store(h_scr, c0):
            ps_tr = psB.tile([128, KC2, 2, 16], dt.float32, space="PSUM", tag="tr")
            for kc in range(H // 128):
                nc.tensor.transpose(out=ps_tr[:, kc // 2, kc % 2, :],
                                    in_=h_scr[:, kc * 128:(kc + 1) * 128],
                                    identity=ident16[:])
            if c0 >= NTOK:
                dst = hT80[:, :, :, :]
            else:
                m, off = c0 // 128, c0 % 128
                dst = hT8m[m][:, :, :, off:off + 16]
            nc.vector.tensor_copy(dst, ps_tr[:])

        emit_h_transpose_store(h_scr0, NTOK)   # h0

        # ---- xW groups (g=0 now; g=1..3 interleaved into early scan) ----
        def emit_xw_group(g):
            ps_x = psC.tile([128, KC_W, 128], dt.float32, space="PSUM", tag="big")
            for wc in range(KC_W):
                nc.tensor.transpose(out=ps_x[:, wc, :],
                                    in_=x_all[:, g, wc * 128:(wc + 1) * 128],
                                    identity=ident128[:])
            xT_g = scr.tile([128, KC_W, 128], dt.float32r, tag="xT")
            nc.vector.tensor_copy(xT_g[:], ps_x[:])
            for hf in range(2):
                ps = psC.tile([128, 512], dt.float32, space="PSUM", tag="big")
                for wc in range(KC_W):
                    nc.tensor.matmul(
                        out=ps[:],
                        lhsT=xT_g[:, wc, :],
                        rhs=wx_t[:, wc, hf * 512:(hf + 1) * 512],
                        start=(wc == 0), stop=(not with_bias and wc == KC_W - 1))
                if with_bias:
                    nc.tensor.matmul(out=ps[:], lhsT=ones128[:],
                                     rhs=brnn_t[:, hf * 512:(hf + 1) * 512],
                                     start=False, stop=True)
                nc.vector.tensor_copy(xw_all[:, g, hf * 512:(hf + 1) * 512], ps[:])

        emit_xw_group(0)

        # ---- paired softmax tile: two 512-col score chains in two PSUM
        # banks, one exp over 1024 cols (ACT ops halve). The epilogue is
        # emitted after the step's tanh so ACT's FIFO serves tanh first.
        def emit_pair_mm(pool, m, j2):
            ps = pool.tile([128, 2, 512], dt.float32, space="PSUM", tag="pair")
            for half in range(2):
                j = 2 * j2 + half
                for kc2 in range(KC2):
                    nc.tensor.matmul(out=ps[:, half, :],
                                     lhsT=hT_lhs(kc2, m * 128, 128),
                                     rhs=wv_tiles[(j, kc2)][:],
                                     start=(kc2 == 0),
                                     stop=(not with_bias and kc2 == KC2 - 1),
                                     perf_mode=mybir.MatmulPerfMode.DoubleRow)
                if with_bias:
                    nc.tensor.matmul(out=ps[:, half, :], lhsT=ones128[:],
                                     rhs=wv_tiles[(j, "b")][:], start=False,
                                     stop=True)
            return ps

        def emit_pair_epi(m, j2, ps):
            exp_s = scr.tile([128, 2, 512], dt.float32, tag="exp")
            nc.scalar.activation(exp_s[:], ps[:], AF.Exp,
                                 accum_out=s_cols[:, m, j2:j2 + 1])

        # interleaved pair schedule: one pair per step once group m is ready
        pair_by_t = {}
        done_in_scan = set()
        for k in range(8):
            pair_by_t[10 + k] = (0, k)
        for k in range(8):
            pair_by_t[18 + k] = (1, k)
        for k in range(7):
            pair_by_t[26 + k] = (2, k)
        done_in_scan.update(pair_by_t.values())

        # ---- phase 2: scan ----
        def emit_inject(ps, b):
            m, i = b // 8, b % 8
            for hf in range(2):
                nc.tensor.matmul(
                    out=ps[:, hf * 512:(hf + 1) * 512],
                    lhsT=ident128r[:, i * 16:i * 16 + 16],
                    rhs=xw_all[:, m, hf * 512:(hf + 1) * 512],
                    start=True, stop=False)

        fillers = {4: 1, 6: 2, 8: 3}
        psP = None
        ps_pending = None
        for t in range(1, T_STEPS + 1):
            b = t - 1                      # token block index [0,32)
            c_prev = NTOK if b == 0 else (b - 1) * 16
            if ps_pending is None:
                ps = psA.tile([NL, H], dt.float32, space="PSUM", tag="scan")
                emit_inject(ps, b)
            else:
                ps = ps_pending
            for hf in range(2):
                for kc2 in range(KC2):
                    nc.tensor.matmul(
                        out=ps[:, hf * 512:(hf + 1) * 512],
                        lhsT=hT_lhs(kc2, c_prev, 16),
                        rhs=wh_t[:, kc2, :, hf * 512:(hf + 1) * 512],
                        start=False, stop=(kc2 == KC2 - 1),
                        perf_mode=mybir.MatmulPerfMode.DoubleRow)
            if t < T_STEPS:
                ps_pending = psA.tile([NL, H], dt.float32, space="PSUM", tag="scan")
                emit_inject(ps_pending, b + 1)
            if t in fillers:
                emit_xw_group(fillers[t])
            if t == 9:
                # xW pool done; its banks become the in-scan pair bank pair
                stack_xw.close()
                psP = stack_scan.enter_context(
                    tc.tile_pool(name="psP", bufs=1, space="PSUM"))
            pend_pair = None
            if t in pair_by_t:
                m_, j2_ = pair_by_t[t]
                pend_pair = (m_, j2_, emit_pair_mm(psP, m_, j2_))
            h_scr = scr.tile([NL, H], dt.float32, tag="hscr")
            for hf in range(2):
                nc.scalar.activation(h_scr[:, hf * 512:(hf + 1) * 512],
                                     ps[:, hf * 512:(hf + 1) * 512], AF.Tanh)
            if pend_pair is not None:
                emit_pair_epi(pend_pair[0], pend_pair[1], pend_pair[2])
            emit_h_transpose_store(h_scr, b * 16)

        # ---- phase 3: scan pools close; deep pair pipeline opens ----
        stack_scan.close()
        psQ = ctx.enter_context(tc.tile_pool(name="psQ", bufs=3, space="PSUM"))
        psD = ctx.enter_context(tc.tile_pool(name="psD", bufs=2, space="PSUM"))

        # target scores: t[tok] = h[tok] . W_vocab[:, y[tok]] via DoubleRow
        # against host-gathered target columns, then diagonal extraction.
        t_diag = acts.tile([128, NG], dt.float32)
        for m in range(NG):
            ps_y = psD.tile([128, 128], dt.float32, space="PSUM", tag="diag")
            for kc2 in range(KC2):
                nc.tensor.matmul(out=ps_y[:],
                                 lhsT=hT8m[m][:, kc2, :, :],
                                 rhs=wvy_t[:, m, kc2, :, :],
                                 start=(kc2 == 0), stop=(kc2 == KC2 - 1),
                                 perf_mode=mybir.MatmulPerfMode.DoubleRow)
            dtrash = scr.tile([128, 128], dt.float32, tag="dtrash")
            nc.vector.scalar_tensor_tensor(
                out=dtrash[:], in0=ident128[:], scalar=1.0, in1=ps_y[:],
                op0=OP.mult, op1=OP.mult, accum_out=t_diag[:, m:m + 1])

        # remaining pairs, ascending j2 so low-j pool slots free early
        for j2 in range(NJ2):
            for m in range(NG):
                if (m, j2) not in done_in_scan:
                    emit_pair_epi(m, j2, emit_pair_mm(psQ, m, j2))

        # ---- tail: ship partials; host does log/mask/sum ----
        nc.sync.dma_start(s_out_d.ap(), s_cols[:])
        nc.sync.dma_start(t_out_d.ap(), t_diag[:])

    nc.compile()
    _nc_cache[with_bias] = nc
    return nc


def make_in_maps(features, captions, W_proj, b_proj, W_embed, Wx, Wh, b,
                 W_vocab, b_vocab, with_bias):
    features = np.asarray(features, dtype=np.float32)
    cap = np.asarray(captions).astype(np.int64)
    Wv = np.asarray(W_vocab, dtype=np.float32)
    wv8 = np.ascontiguousarray(
        Wv.reshape(KC2, 2, 128, NJ, 512).transpose(0, 3, 2, 1, 4)).astype(E4)
    wh8 = np.ascontiguousarray(
        np.asarray(Wh, dtype=np.float32)
        .reshape(KC2, 2, 128, H).transpose(0, 2, 1, 3)).astype(E4)
    # DR-layout view of Wv columns: [kc2, i, p, v] for target-column gathers
    wv_dr_cols = Wv.reshape(KC2, 2, 128, V)
    shared = {
        "W_embed": np.asarray(W_embed, dtype=np.float32),
        "W_proj": np.asarray(W_proj, dtype=np.float32),
        "Wx": np.asarray(Wx, dtype=np.float32),
        "Wh8": wh8,
        "WV8": wv8,
    }
    if with_bias:
        shared.update({
            "b_proj": np.asarray(b_proj, dtype=np.float32).reshape(1, H),
            "b_rnn": np.asarray(b, dtype=np.float32).reshape(1, H),
            "b_vocab": np.asarray(b_vocab, dtype=np.float32).reshape(NJ, 512),
        })
    in_maps = []
    for c in range(N_CORES):
        capc = cap[c * NL:(c + 1) * NL]              # (16, 33)
        tok_tm = capc[:, :T_STEPS].T.reshape(NTOK)   # token ids, t-major
        y_tm = capc[:, 1:].T.reshape(NTOK)           # targets, t-major
        tok_pg = tok_tm.reshape(NG, 128).T.astype(np.int32).copy()   # (128, NG)
        y_pg = y_tm.reshape(NG, 128).T                               # (128, NG)
        # target columns of W_vocab in DR layout: [m][p][kc2][i][tok 128]
        wvy8 = np.ascontiguousarray(
            wv_dr_cols[:, :, :, y_pg.T.reshape(NG, 128)]             # kc2,i,p,m,n
            .transpose(3, 2, 0, 1, 4)).astype(E4)                    # m,p,kc2,i,n
        in_maps.append({
            "features": features[c * NL:(c + 1) * NL],
            "tok": tok_pg,
            "WVY8": wvy8,
            **shared,
        })
    return in_maps


def finish_loss(results, captions, b_vocab=None):
    cap = np.asarray(captions).astype(np.int64)
    bv = None if b_vocab is None else np.asarray(b_vocab, np.float64)
    total = 0.0
    for c in range(N_CORES):
        y_tm = cap[c * NL:(c + 1) * NL][:, 1:].T.reshape(NTOK)
        y_pg = y_tm.reshape(NG, 128).T                               # (128, NG)
        mask = (y_pg != 0)
        s_red = np.asarray(results[c]["s_out"], np.float64).sum(-1)  # (128, NG)
        t_red = np.asarray(results[c]["t_out"], np.float64)          # (128, NG)
        if bv is not None:
            t_red = t_red + bv[y_pg]
        nll = np.log(s_red) - t_red
        total += float((nll * mask).sum())
    return np.float32(total / 128.0)


def kernel(**inputs) -> np.ndarray:
    with_bias = bool(
        np.any(np.asarray(inputs["b_proj"])) or np.any(np.asarray(inputs["b"]))
        or np.any(np.asarray(inputs["b_vocab"])))
    nc = build_program(with_bias)
    in_maps = make_in_maps(**inputs, with_bias=with_bias)
    res = run_bass_kernel_spmd(nc, in_maps, list(range(N_CORES)))
    return finish_loss(res.results, inputs["captions"],
                       inputs["b_vocab"] if with_bias else None)
